# revision 1
# baseline (speedup 1.0000x reference)
"""Distributed Bass kernel for nn_Attention_32701880992127 on 8 TRN2 NeuronCores.

Sharding (tensor parallel over heads): core d owns q-heads {2d, 2d+1} and
kv-head d//2 (GQA consecutive-repeat mapping). wq/wk/wv are column-split,
wo is split along its OUTPUT dim so each core produces a distinct column
slice of the final output from the AllGathered attention features (cheaper
than the all-reduce variant: 1MB/core gathered vs 16MB reduced).

All matmuls run in bf16 (f32 PSUM accumulation); elementwise math stays f32.
Softmax needs no max-subtraction (qk-norm bounds the scores), and the sink
correction folds into the denominator:
    out_h = (sum_k exp(s_qk) v_k) / (exp(sink_h) + sum_k exp(s_qk)).
Scores are computed transposed (ST[k, q]) so exp's output directly feeds the
PV matmul as the moving operand (no P transposes). The k-side RMS scale and
1/sqrt(dh) are folded into k-hat before its transpose, so exp needs no
per-chunk scale and two k-chunks share one [128, 1024] Exp call. The softmax
denominator is an ones-column (M=1) colsum matmul into a [1, q] PSUM row;
exp(sink) is added on DVE before a tiny reciprocal, and the row is
re-broadcast with a K=1 matmul. The AllGather is chunked along q (4
collectives) to overlap with attention and the output projection.
"""
import numpy as np
import ml_dtypes

import concourse.mybir as mybir
import concourse.tile as tile
from concourse import bacc
from concourse.bass_utils import run_bass_kernel_spmd
from concourse.masks import make_identity

dt = mybir.dt
AO = mybir.AluOpType
AF = mybir.ActivationFunctionType
BF16 = ml_dtypes.bfloat16

N_CORES = 8
S = 2048            # sequence length
D = 2048            # model dim
DH = 128            # head dim
HL = 2              # local q heads per core
NC = 16             # d-chunks of 128
NST = 16            # s-tiles of 128
QT = 512            # attention q tile
NQT = S // QT
KC = 128            # attention k chunk
RMS_EPS = 1.1920929e-07
SQRT_DH = float(np.sqrt(DH))
MAGIC = 0x5F3759DF


def _rsqrt_newton(nc, rs, ssq, tn, hn):
    """rs = rsqrt(ssq) elementwise via bit trick + 2 Newton iterations."""
    nc.vector.tensor_scalar(out=rs.bitcast(dt.int32), in0=ssq.bitcast(dt.int32),
                            scalar1=1, scalar2=None, op0=AO.logical_shift_right)
    nc.vector.tensor_scalar(out=rs.bitcast(dt.int32), in0=rs.bitcast(dt.int32),
                            scalar1=MAGIC, scalar2=-1, op0=AO.subtract, op1=AO.mult)
    nc.vector.tensor_scalar(out=hn, in0=ssq, scalar1=0.5, scalar2=None, op0=AO.mult)
    for _ in range(2):
        nc.vector.tensor_tensor(out=tn, in0=rs, in1=rs, op=AO.mult)
        nc.vector.tensor_tensor(out=tn, in0=tn, in1=hn, op=AO.mult)
        nc.vector.tensor_scalar(out=tn, in0=tn, scalar1=1.5, scalar2=-1.0,
                                op0=AO.subtract, op1=AO.mult)
        nc.vector.tensor_tensor(out=rs, in0=rs, in1=tn, op=AO.mult)


def build():
    nc = bacc.Bacc("TRN2", target_bir_lowering=False, debug=False, num_devices=N_CORES)

    xt = nc.dram_tensor("xt", [D, S], dt.bfloat16, kind="ExternalInput").ap()
    wqkv = nc.dram_tensor("wqkv", [D, 512], dt.bfloat16, kind="ExternalInput").ap()
    wot = nc.dram_tensor("wot", [D, HL * DH], dt.bfloat16, kind="ExternalInput").ap()
    # cbar/sbar: pair-interleave-expanded cos/sin, duplicated for both heads [S, 256]
    cbar = nc.dram_tensor("cbar", [S, HL * DH], dt.float32, kind="ExternalInput").ap()
    sbar = nc.dram_tensor("sbar", [S, HL * DH], dt.float32, kind="ExternalInput").ap()
    # 2 pair-masks [128, 1024] each, stacked: [256, 1024]
    masks = nc.dram_tensor("masks", [2 * KC, 2 * QT], dt.bfloat16, kind="ExternalInput").ap()
    esrow = nc.dram_tensor("esrow", [HL, QT], dt.bfloat16, kind="ExternalInput").ap()
    y_out = nc.dram_tensor("y", [S, HL * DH], dt.float32, kind="ExternalOutput").ap()

    with tile.TileContext(nc) as tc:
        with (
            tc.tile_pool(name="const", bufs=1) as cp,
            tc.tile_pool(name="work", bufs=2) as wp,
            tc.tile_pool(name="psum", bufs=2, space="PSUM") as pp,
            tc.tile_pool(name="dram", bufs=1, space="DRAM") as dp,
        ):
            # ---- constant / persistent tiles (xt first: it gates the projections) ----
            # wqkv first (small, gates the first matmul), then xt by s-groups so
            # the first projection s-tiles can start before the whole 8MB lands
            wqkv_sb = cp.tile([128, NC, 512], dt.bfloat16, tag="wqkv")
            wqr = wqkv.rearrange("(c p) e -> p c e", p=128)
            for c in range(0, NC, 4):
                eng = nc.sync if c % 8 == 0 else nc.scalar
                eng.dma_start(wqkv_sb[:, c:c + 4, :], wqr[:, c:c + 4, :])
            xt_sb = cp.tile([128, NC, S], dt.bfloat16, tag="xt")
            xtr = xt.rearrange("(c p) s -> p c s", p=128)
            for g in range(4):
                gsl = slice(g * 512, (g + 1) * 512)
                nc.sync.dma_start(xt_sb[:, 0:8, gsl], xtr[:, 0:8, gsl])
                nc.scalar.dma_start(xt_sb[:, 8:16, gsl], xtr[:, 8:16, gsl])
            wot_sb = cp.tile([128, NC, HL * DH], dt.bfloat16, tag="wot")
            cbar_sb = cp.tile([128, NST, HL * DH], dt.float32, tag="cbar")
            cbr = cbar.rearrange("(c p) e -> p c e", p=128)
            sbar_sb = cp.tile([128, NST, HL * DH], dt.float32, tag="sbar")
            sbr = sbar.rearrange("(c p) e -> p c e", p=128)
            for c in range(0, NST, 4):
                nc.scalar.dma_start(cbar_sb[:, c:c + 4, :], cbr[:, c:c + 4, :])
                nc.scalar.dma_start(sbar_sb[:, c:c + 4, :], sbr[:, c:c + 4, :])
            mask_sb = cp.tile([128, 2, 2 * QT], dt.bfloat16, tag="mask")
            es_sb = []
            for h in range(HL):
                t_es = cp.tile([1, QT], dt.bfloat16, tag=f"es{h}")
                nc.scalar.dma_start(t_es[:], esrow[h:h + 1, :])
                es_sb.append(t_es)

            ident = cp.tile([128, 128], dt.bfloat16, tag="ident")
            make_identity(nc, ident[:])
            ones128 = cp.tile([128, 128], dt.bfloat16, tag="ones128")
            nc.vector.memset(ones128[:], 1.0)
            ones1 = cp.tile([1, 128], dt.bfloat16, tag="ones1")
            nc.vector.memset(ones1[:], 1.0)

            qT = cp.tile([128, HL, S], dt.bfloat16, tag="qT")       # normed+roped q [dh, h, s]
            kT = cp.tile([128, S], dt.bfloat16, tag="kT")           # roped+scaled k [dh, s]
            v_sb = cp.tile([128, NST, DH], dt.bfloat16, tag="v")    # v natural [s-tile][128, dh]
            attnT = cp.tile([128, HL, S], dt.bfloat16, tag="attnT")  # attn out [dh, h, q]

            # ---- AllGather bounce buffers (one pair per q-slice) ----
            ag_ins = [dp.tile([HL * 128, QT], dt.bfloat16, name=f"ag_in{i}")
                      for i in range(NQT)]
            ag_outs = [dp.tile([N_CORES * HL * 128, QT], dt.bfloat16, addr_space="Shared",
                               name=f"ag_out{i}") for i in range(NQT)]

            def attention_group(t):
                qsl = slice(t * QT, (t + 1) * QT)
                npairs = 2 * (t + 1)
                for h in range(HL):
                    lacc = pp.tile([128, QT], dt.float32, tag="lacc", bufs=1)
                    oacc = pp.tile([128, QT], dt.float32, tag="oacc", bufs=1)
                    # denominator starts with exp(sink), broadcast to all partitions
                    nc.tensor.matmul(lacc[:], ones1[:], es_sb[h][:], start=True, stop=False)
                    for p in range(npairs):
                        c0, c1 = 2 * p, 2 * p + 1
                        stp = pp.tile([128, 2 * QT], dt.float32, tag="stp")
                        nc.tensor.matmul(stp[:, 0:QT], kT[:, c0 * KC:(c0 + 1) * KC],
                                         qT[:, h, qsl], start=True, stop=True)
                        nc.tensor.matmul(stp[:, QT:2 * QT], kT[:, c1 * KC:(c1 + 1) * KC],
                                         qT[:, h, qsl], start=True, stop=True)
                        pt = wp.tile([128, 2 * QT], dt.bfloat16, tag="pt", bufs=4)
                        nc.scalar.activation(pt[:], stp[:], AF.Exp)
                        if p >= 2 * t:
                            nc.vector.tensor_tensor(out=pt[:], in0=pt[:],
                                                    in1=mask_sb[:, p - 2 * t, :], op=AO.mult)
                        nc.tensor.matmul(lacc[:], ones128[:], pt[:, 0:QT],
                                         start=False, stop=False)
                        nc.tensor.matmul(lacc[:], ones128[:], pt[:, QT:2 * QT],
                                         start=False, stop=(p == npairs - 1))
                        nc.tensor.matmul(oacc[:], v_sb[:, c0, :], pt[:, 0:QT],
                                         start=(p == 0), stop=False)
                        nc.tensor.matmul(oacc[:], v_sb[:, c1, :], pt[:, QT:2 * QT],
                                         start=False, stop=(p == npairs - 1))
                    # out = oacc / (l + exp(sink))
                    rr = wp.tile([128, QT], dt.float32, tag="rr")
                    nc.vector.reciprocal_approx_fast(rr[:], lacc[:])
                    nc.vector.tensor_tensor(out=attnT[:, h, qsl], in0=oacc[:], in1=rr[:],
                                            op=AO.mult)
                    # ship this head's q-slice immediately (half of the AG input)
                    nc.scalar.dma_start(
                        ag_ins[t][:].rearrange("(h p) q -> p h q", p=128)[:, h, :],
                        attnT[:, h, qsl])
                if True:
                    nc.gpsimd.collective_compute(
                        "AllGather", AO.bypass,
                        replica_groups=[list(range(N_CORES))],
                        ins=[ag_ins[t][:].opt()], outs=[ag_outs[t][:].opt()],
                    )

            def wo_part(t):
                agr = ag_outs[t][:].rearrange("(c p) q -> p c q", p=128)
                for tt in range(QT // 128):
                    aT = wp.tile([128, NC, 128], dt.bfloat16, tag="aT", bufs=4)
                    nc.sync.dma_start(aT[:], agr[:, :, tt * 128:(tt + 1) * 128])
                    yp = pp.tile([128, 512], dt.float32, tag="mm")
                    for c in range(NC):
                        nc.tensor.matmul(yp[:, 0:HL * DH], aT[:, c, :], wot_sb[:, c, :],
                                         start=(c == 0), stop=(c == NC - 1))
                    ysb = wp.tile([128, HL * DH], dt.float32, tag="ysb")
                    nc.scalar.copy(ysb[:], yp[:, 0:HL * DH])
                    nc.sync.dma_start(y_out[t * QT + tt * 128:t * QT + (tt + 1) * 128, :],
                                      ysb[:])

            # ---- interleaved emission: projections + attention groups ----
            for st in range(NST):
                ssl = slice(st * 128, (st + 1) * 128)
                mm = pp.tile([128, 512], dt.float32, tag="mm")  # q[0:256] | k[256:384] | v[384:512]
                for c in range(NC):
                    nc.tensor.matmul(mm[:], xt_sb[:, c, ssl], wqkv_sb[:, c, :],
                                     start=(c == 0), stop=(c == NC - 1))

                if st == 2:
                    wotr = wot.rearrange("(c p) e -> p c e", p=128)
                    for c in range(0, NC, 4):
                        nc.sync.dma_start(wot_sb[:, c:c + 4, :], wotr[:, c:c + 4, :])
                    nc.sync.dma_start(mask_sb[:], masks.rearrange("(j p) q -> p j q", p=128))

                # evacuate PSUM quickly: q|k to f32 SBUF (ACT), v to bf16 (ACT)
                qk = wp.tile([128, 384], dt.float32, tag="qk", bufs=4)
                nc.vector.tensor_copy(qk[:], mm[:, 0:384])
                nc.vector.tensor_copy(v_sb[:, st, :], mm[:, 384:512])

                # sum of squares for q heads and k (ACT Square, same table set as Exp)
                ssq = wp.tile([128, 4], dt.float32, tag="ssq")
                scr = wp.tile([128, 128], dt.float32, tag="scr")
                for i in range(3):
                    nc.scalar.activation(scr[:], qk[:, i * DH:(i + 1) * DH], AF.Square,
                                         accum_out=ssq[:, i:i + 1])

                # rs = rsqrt(ssq + 128*eps); cols 0,1 = q heads, col 2 = k
                rs = wp.tile([128, 4], dt.float32, tag="rs")
                tn = wp.tile([128, 4], dt.float32, tag="tn")
                hn = wp.tile([128, 4], dt.float32, tag="hn")
                nc.vector.tensor_scalar(out=ssq[:], in0=ssq[:], scalar1=128.0 * RMS_EPS,
                                        scalar2=None, op0=AO.add)
                _rsqrt_newton(nc, rs[:], ssq[:], tn[:], hn[:])
                # q scale: rsqrt(mean+eps) = rs * sqrt(128); k keeps rs (has 1/sqrt(dh) folded)
                nc.vector.tensor_scalar(out=rs[:, 0:2], in0=rs[:, 0:2], scalar1=SQRT_DH,
                                        scalar2=None, op0=AO.mult)

                # rope q (both heads in one set of ops; 3-D APs pair the heads)
                q3e = qk[:, 0:256].rearrange("p (h d) -> p h d", h=HL)[:, :, 0:DH:2]
                q3o = qk[:, 0:256].rearrange("p (h d) -> p h d", h=HL)[:, :, 1:DH:2]
                w = wp.tile([128, HL * DH], dt.float32, tag="w")
                w3 = w[:].rearrange("p (h d) -> p h d", h=HL)
                nc.vector.tensor_scalar(out=w3[:, :, 0:DH:2], in0=q3o, scalar1=-1.0,
                                        scalar2=None, op0=AO.mult)
                nc.vector.tensor_copy(w3[:, :, 1:DH:2], q3e)
                u1 = wp.tile([128, HL * DH], dt.float32, tag="u1")
                qhat = wp.tile([128, HL * DH], dt.bfloat16, tag="qhat")
                nc.vector.tensor_tensor(out=u1[:], in0=qk[:, 0:256], in1=cbar_sb[:, st, :],
                                        op=AO.mult)
                nc.vector.tensor_tensor(out=w[:], in0=w[:], in1=sbar_sb[:, st, :], op=AO.mult)
                nc.vector.tensor_add(out=qhat[:], in0=u1[:], in1=w[:])
                for h in range(HL):
                    nc.vector.tensor_scalar(out=qhat[:, h * DH:(h + 1) * DH],
                                            in0=qhat[:, h * DH:(h + 1) * DH],
                                            scalar1=rs[:, h:h + 1], scalar2=None, op0=AO.mult)

                # rope k on gpsimd (rk scale folded in afterwards)
                kw = wp.tile([128, DH], dt.float32, tag="kw")
                ku = wp.tile([128, DH], dt.float32, tag="ku")
                khat = wp.tile([128, DH], dt.bfloat16, tag="khat")
                nc.vector.tensor_scalar(out=kw[:, 0:DH:2], in0=qk[:, 256 + 1:384:2],
                                        scalar1=-1.0, scalar2=None, op0=AO.mult)
                nc.vector.tensor_copy(kw[:, 1:DH:2], qk[:, 256 + 0:384:2])
                nc.vector.tensor_tensor(out=ku[:], in0=qk[:, 256:384],
                                        in1=cbar_sb[:, st, 0:DH], op=AO.mult)
                nc.vector.tensor_tensor(out=kw[:], in0=kw[:], in1=sbar_sb[:, st, 0:DH],
                                        op=AO.mult)
                nc.vector.tensor_add(out=ku[:], in0=ku[:], in1=kw[:])
                nc.vector.tensor_scalar(out=khat[:], in0=ku[:], scalar1=rs[:, 2:3],
                                        scalar2=None, op0=AO.mult)

                # transposes -> qT / kT (PSUM copies on ACT)
                for h in range(HL):
                    tp = pp.tile([128, 128], dt.bfloat16, tag="stp")
                    nc.tensor.transpose(tp[:], qhat[:, h * DH:(h + 1) * DH], ident[:])
                    nc.scalar.copy(qT[:, h, ssl], tp[:])
                tpk = pp.tile([128, 128], dt.bfloat16, tag="stp")
                nc.tensor.transpose(tpk[:], khat[:], ident[:])
                nc.scalar.copy(kT[:, ssl], tpk[:])

                if st % 4 == 3:
                    attention_group(st // 4)
                    if st // 4 >= 2:
                        wo_part(st // 4 - 2)

            wo_part(NQT - 2)
            wo_part(NQT - 1)

    nc.compile()
    return nc


def prep_inputs(x, freqs_cis, wq, wk, wv, wo, sinks):
    """Host-side sharding/layout prep. Returns in_maps for the 8 cores."""
    x2 = np.ascontiguousarray(np.asarray(x, np.float32).reshape(S, D))
    xt = np.ascontiguousarray(x2.T).astype(BF16)
    fc = np.asarray(freqs_cis, np.float32)
    cos, sin = fc[:, :, 0], fc[:, :, 1]
    # pair-interleaved expansion duplicated for 2 heads: cbar[s, h*128 + 2j(+1)] = cos[s, j]
    cbar1 = np.repeat(cos, 2, axis=1)          # [S, 128]
    sbar1 = np.repeat(sin, 2, axis=1)
    cbar = np.tile(cbar1, (1, HL)).astype(np.float32)
    sbar = np.tile(sbar1, (1, HL)).astype(np.float32)

    # 2 pair-masks [128, 1024]: pair j covers chunks (2j, 2j+1) at the diagonal
    kr = np.arange(KC)[:, None]
    qr = np.arange(QT)[None, :]
    m4 = [(qr >= kr + KC * j).astype(np.float32) for j in range(4)]
    mp0 = np.concatenate([m4[0], m4[1]], axis=1)   # [128, 1024]
    mp1 = np.concatenate([m4[2], m4[3]], axis=1)
    masks = np.concatenate([mp0, mp1], axis=0).astype(BF16)  # [256, 1024]

    wq = np.asarray(wq, np.float32)
    wk = np.asarray(wk, np.float32)
    wv = np.asarray(wv, np.float32)
    wo = np.asarray(wo, np.float32)
    sinks = np.asarray(sinks, np.float32)

    in_maps = []
    for d in range(N_CORES):
        kv = d // 2
        es = np.exp(sinks[2 * d:2 * d + 2]).astype(np.float32)
        wqkv = np.concatenate([
            wq[d * 256:(d + 1) * 256, :].T,
            wk[kv * 128:(kv + 1) * 128, :].T,
            wv[kv * 128:(kv + 1) * 128, :].T,
        ], axis=1)
        in_maps.append({
            "xt": xt,
            "wqkv": np.ascontiguousarray(wqkv).astype(BF16),
            "wot": np.ascontiguousarray(wo[d * 256:(d + 1) * 256, :].T).astype(BF16),
            "cbar": cbar,
            "sbar": sbar,
            "masks": masks,
            "esrow": np.repeat(es[:, None], QT, axis=1).astype(BF16),
        })
    return in_maps


_CACHED = {}


def kernel(x, freqs_cis, wq, wk, wv, wo, sinks):
    if "nc" not in _CACHED:
        _CACHED["nc"] = build()
    nc = _CACHED["nc"]
    in_maps = prep_inputs(x, freqs_cis, wq, wk, wv, wo, sinks)
    res = run_bass_kernel_spmd(nc, in_maps, list(range(N_CORES)), trace=False)
    y = np.concatenate([res.results[d]["y"] for d in range(N_CORES)], axis=1)
    return y.reshape(1, S, D).astype(np.float32)

